# revision 23
# baseline (speedup 1.0000x reference)
"""Trainium2 Bass kernel for PVT-style spatial-reduction attention with LoRA.

Sharding: 8 cores = (batch b in {0,1}) x (query-token quarter qi in {0..3}).
Each core computes the full spatial-reduction conv + LayerNorm + K/V
(replicated within a batch group) and attention + output projection for its
own 1024 query tokens. No collectives at all: the host concatenates the
per-core output slices. The whole per-rep computation sits inside a For_i
hardware loop, so multi-rep NEFFs stay the same static size as reps=1.

All activations live transposed ([feature, token]) on device. Host folds:
LoRA into the dense weights, softmax scale into Wq/bq, LN gamma into Wk/Wv.
The per-position LN shift/scale is applied as xsn = xs*rstd (matmul-broadcast
of rstd along partitions) plus rank-1 correction matmuls (wg1 x (-mu*rstd)
and beta-derived column biases) accumulated directly into the K/V PSUM
groups. Softmax denominators come from an all-ones column appended to each
head's V block; max-subtraction is skipped (logits are bounded ~|2|).
"""
import sys
for _p in ('/opt/trn_rl_repo', '/root/.axon_site/_ro/trn_rl_repo'):
    if _p not in sys.path:
        sys.path.insert(0, _p)

import numpy as np

B, N, C, HEAD, SR, R = 2, 4096, 512, 8, 2, 8
HH = WW = 64
DH = C // HEAD               # 64
M = (HH // SR) * (WW // SR)  # 1024 kv positions
NQ = N // 4                  # 1024 query tokens per core
LN_EPS = 1e-5
NCORES = 8

_cached = {}

# sm (small-vector) column layout
SM_WG1K = 0
SM_WG1V = 512
SM_CBK = 1024
SM_CBV = 1536
SM_BFIN = 2048
SM_ONES = 2560
SM_LEN = 3584


def _build_nc(reps=1):
    from concourse import bacc, tile, mybir
    import concourse.bass as bass_mod

    f32 = mybir.dt.float32
    f16 = mybir.dt.float16
    ACT = mybir.ActivationFunctionType

    nc = bacc.Bacc("TRN2", target_bir_lowering=False, debug=False,
                   num_devices=NCORES)
    xTs_d = nc.dram_tensor("xTs", [C, N], f16, kind="ExternalInput")
    xq_d = nc.dram_tensor("xq", [C, NQ], f16, kind="ExternalInput")
    wsr_d = nc.dram_tensor("wsr", [16, 128, C], f16, kind="ExternalInput")
    wbig_d = nc.dram_tensor("wbig", [4, 128, 4 * C], f16, kind="ExternalInput")
    sm_d = nc.dram_tensor("sm", [1, SM_LEN], f16, kind="ExternalInput")
    sm2_d = nc.dram_tensor("sm2", [2, 2 * C], f16, kind="ExternalInput")
    vecs_d = nc.dram_tensor("vecs", [128, 8], f32, kind="ExternalInput")
    cst_d = nc.dram_tensor("cst", [128, 1], f16, kind="ExternalInput")
    eps_d = nc.dram_tensor("epsc", [1, 1], f32, kind="ExternalInput")
    out_d = nc.dram_tensor("outT", [C, NQ], f16, kind="ExternalOutput")

    with tile.TileContext(nc) as tc:
        with tc.tile_pool(name="sb", bufs=1) as sb, \
             tc.tile_pool(name="pex", bufs=4) as pex, \
             tc.tile_pool(name="fin", bufs=2) as fin, \
             tc.tile_pool(name="dbl", bufs=2) as dbl, \
             tc.tile_pool(name="ps", bufs=1, space="PSUM") as ps:

            xT = sb.tile([128, 4, N], f16)
            xq = sb.tile([128, 4, NQ], f16)
            wsr = sb.tile([128, 16, C], f16)
            wbig = sb.tile([128, 4, 4 * C], f16)
            sm = sb.tile([1, SM_LEN], f16)
            sm2 = sb.tile([2, 2 * C], f16)
            vecs = sb.tile([128, 8], f32)
            cst = sb.tile([128, 1], f16)
            epsc = sb.tile([1, 1], f32)
            xs = sb.tile([128, 4, M], f16)
            sq = sb.tile([128, 4, M], f16)
            mean_s = sb.tile([1, M], f32)
            e2_s = sb.tile([1, M], f32)
            msq_s = sb.tile([1, M], f32)
            rstd16 = sb.tile([1, M], f16)
            stat2 = sb.tile([2, M], f16)
            xsn = sb.tile([128, 4, M], f16)
            outT = sb.tile([128, 4, NQ], f16)
            on64 = sb.tile([65, 64], f16)

            ones = lambda n: sm[0:1, SM_ONES:SM_ONES + n]

            # ---- weights / constants: loaded once, stationary across reps --
            nc.sync.dma_start(wsr[:], wsr_d.rearrange("g p n -> p g n"))
            nc.sync.dma_start(wbig[:], wbig_d.rearrange("t p n -> p t n"))
            nc.sync.dma_start(sm[:], sm_d[:])
            nc.sync.dma_start(sm2[:], sm2_d[:])
            nc.sync.dma_start(vecs[:], vecs_d[:])
            nc.sync.dma_start(cst[:], cst_d[:])
            nc.sync.dma_start(epsc[:], eps_d[:])
            nc.vector.memset(on64[64:65, :], 1.0)
            nc.sync.dma_start(stat2[1:2, :],
                              sm_d[0:1, SM_ONES:SM_ONES + M])

            def emit_rep():
                kT = dbl.tile([128, 4, M], f16, tag="kT")
                vt = dbl.tile([128, 8, 520], f16, tag="vt")
                qT = dbl.tile([128, 4, NQ], f16, tag="qT")
                att = dbl.tile([128, 4, NQ], f16, tag="att")
                # ---- load activations (split across DMA queues) ----
                xqv = xq_d.rearrange("(t p) n -> p t n", p=128)
                for t in range(2):
                    nc.sync.dma_start(xq[:, 2 * t:2 * t + 2, :],
                                      xqv[:, 2 * t:2 * t + 2, :])
                xTv = xTs_d.rearrange("(t p) n -> p t n", p=128)
                for t in range(4):
                    nc.sync.dma_start(xT[:, t, :], xTv[:, t, :])

                # ---- qT [C, NQ] (this core's token quarter; input lands
                # first, keeps PE busy while xT streams in) ----
                for jb in range(4):
                    for th in range(2):
                        acc = ps.tile([128, 512], f32, tag="nrw", bufs=4,
                                      name="qacc")
                        for ct in range(4):
                            nc.tensor.matmul(
                                acc[:], wbig[:, ct, jb * 128:(jb + 1) * 128],
                                xq[:, ct, th * 512:(th + 1) * 512],
                                start=(ct == 0), stop=(ct == 3))
                        nc.scalar.activation(
                            out=qT[:, jb, th * 512:(th + 1) * 512], in_=acc[:],
                            func=ACT.Identity, bias=vecs[:, 4 + jb:5 + jb],
                            scale=1.0)

                # ---- conv: xs_pre^T [C, M] ----
                xview = xT.rearrange("p t (ph a pw b) -> p t ph a pw b",
                                     ph=32, a=2, pw=32, b=2)
                for cb in range(4):
                    for mc in range(2):
                        acc = ps.tile([128, 512], f32, tag="nrw", bufs=4,
                                      name="cacc")
                        for g in range(16):
                            dydx, ct = g // 4, g % 4
                            dy, dx = dydx // 2, dydx % 2
                            rhs = xview[:, ct, mc * 16:(mc + 1) * 16, dy, :, dx]
                            nc.tensor.matmul(
                                acc[:], wsr[:, g, cb * 128:(cb + 1) * 128],
                                rhs, start=(g == 0), stop=(g == 15))
                        nc.scalar.activation(
                            out=xs[:, cb, mc * 512:(mc + 1) * 512], in_=acc[:],
                            func=ACT.Identity, bias=vecs[:, cb:cb + 1],
                            scale=1.0)

                # ---- LN stats ----
                nc.vector.tensor_mul(sq[:], xs[:], xs[:])
                for src, dst in ((xs, mean_s), (sq, e2_s)):
                    for mh in range(2):
                        st = ps.tile([1, 512], f32, tag="nrw", bufs=4,
                                     name="st")
                        for cb in range(4):
                            nc.tensor.matmul(
                                st[:], cst[:, 0:1],
                                src[:, cb, mh * 512:(mh + 1) * 512],
                                start=(cb == 0), stop=(cb == 3))
                        nc.vector.tensor_copy(
                            dst[0:1, mh * 512:(mh + 1) * 512], st[:])
                nc.vector.tensor_mul(msq_s[:], mean_s[:], mean_s[:])
                nc.vector.tensor_sub(e2_s[:], e2_s[:], msq_s[:])
                nc.scalar.activation(out=e2_s[:], in_=e2_s[:], func=ACT.Sqrt,
                                     bias=epsc[0:1, 0:1], scale=1.0)
                nc.vector.reciprocal(e2_s[:], e2_s[:])       # rstd
                nc.vector.tensor_mul(mean_s[:], mean_s[:], e2_s[:])
                nc.scalar.mul(mean_s[:], mean_s[:], -1.0)    # -mu*rstd
                nc.vector.tensor_copy(rstd16[:], e2_s[:])
                nc.vector.tensor_copy(stat2[0:1, :], mean_s[:])

                # xsn = xs * rstd (broadcast rstd along partitions via matmul)
                rbc = ps.tile([128, M], f32, tag="wide", bufs=2, name="rbc")
                for mh in range(2):
                    nc.tensor.matmul(rbc[:, mh * 512:(mh + 1) * 512],
                                     ones(128),
                                     rstd16[0:1, mh * 512:(mh + 1) * 512],
                                     start=True, stop=True)
                for cb in range(4):
                    nc.vector.tensor_mul(xsn[:, cb, :], xs[:, cb, :], rbc[:])

                # ---- kT [C, M] ----
                for jb in range(4):
                    for mh in range(2):
                        acc = ps.tile([128, 512], f32, tag="nrw", bufs=4,
                                      name="kacc")
                        for ct in range(4):
                            nc.tensor.matmul(
                                acc[:],
                                wbig[:, ct, C + jb * 128:C + (jb + 1) * 128],
                                xsn[:, ct, mh * 512:(mh + 1) * 512],
                                start=(ct == 0), stop=False)
                        nc.tensor.matmul(
                            acc[:], sm2[0:2, jb * 128:(jb + 1) * 128],
                            stat2[0:2, mh * 512:(mh + 1) * 512],
                            start=False, stop=True)
                        nc.vector.tensor_copy(
                            kT[:, jb, mh * 512:(mh + 1) * 512], acc[:])

                # ---- v [M, C] + per-head ones column ----
                nc.vector.memset(bass_mod.AP(
                    tensor=vt.tensor, offset=vt.offset + 64,
                    ap=[list(vt.ap[0]), [520, 8], [65, 8]]), 1.0)
                for kt in range(8):
                    acc = ps.tile([128, 512], f32, tag="nrw", bufs=4,
                                  name="vacc")
                    for ct in range(4):
                        nc.tensor.matmul(
                            acc[:], xsn[:, ct, kt * 128:(kt + 1) * 128],
                            wbig[:, ct, 2 * C:3 * C],
                            start=(ct == 0), stop=False)
                    nc.tensor.matmul(
                        acc[:], stat2[0:2, kt * 128:(kt + 1) * 128],
                        sm2[0:2, C:C + 512],
                        start=False, stop=True)
                    vdst = bass_mod.AP(tensor=vt.tensor,
                                       offset=vt.offset + kt * 520,
                                       ap=[list(vt.ap[0]), [65, 8], [1, 64]])
                    nc.vector.tensor_copy(
                        vdst, acc.rearrange("p (h d) -> p h d", h=8))

                # ---- attention ----
                for h in range(8):
                    hb, ho = h // 2, 64 * (h % 2)
                    for th in range(2):
                        ops = ps.tile([65, 512], f32, tag="nrw", bufs=4,
                                      name="ops")
                        # scores + exp for all chunks first, then the av
                        # accumulation: keeps independent score matmuls ahead
                        # of exp-dependent av matmuls in the PE queue.
                        pexps = []
                        for mc2 in range(4):
                            sps = ps.tile([128, 1024], f32, tag="wide",
                                          bufs=2, name="sps")
                            for half in range(2):
                                mc = 2 * mc2 + half
                                nc.tensor.matmul(
                                    sps[:, half * 512:(half + 1) * 512],
                                    kT[ho:ho + 64, hb,
                                       mc * 128:(mc + 1) * 128],
                                    qT[ho:ho + 64, hb,
                                       th * 512:(th + 1) * 512],
                                    start=True, stop=True)
                            pexp = pex.tile([128, 1024], f16, tag="pexp")
                            nc.scalar.activation(out=pexp[:], in_=sps[:],
                                                 func=ACT.Exp)
                            pexps.append(pexp)
                        for mc2 in range(4):
                            for half in range(2):
                                mc = 2 * mc2 + half
                                nc.tensor.matmul(
                                    ops[:], vt[:, mc, 65 * h:65 * h + 65],
                                    pexps[mc2][:, half * 512:(half + 1) * 512],
                                    start=(mc == 0), stop=(mc == 7))
                        rbt = fin.tile([65, 512], f16, tag="rbt", name="rbt")
                        with nc.allow_low_precision(reason="f16 recip ok"):
                            nc.vector.reciprocal(rbt[64:65, :], ops[64:65, :])
                        bcd = ps.tile([64, 512], f32, tag="nrw", bufs=4,
                                      name="bcd")
                        nc.tensor.matmul(bcd[:], on64[64:65, :],
                                         rbt[64:65, :],
                                         start=True, stop=True)
                        num = fin.tile([64, 512], f16, tag="num", name="num")
                        nc.vector.tensor_copy(num[:], ops[0:64, :])
                        nc.vector.tensor_mul(
                            att[ho:ho + 64, hb, th * 512:(th + 1) * 512],
                            num[:], bcd[:])

                # ---- out proj ----
                for jb in range(4):
                    for th in range(2):
                        acc = ps.tile([128, 512], f32, tag="nrw", bufs=4,
                                      name="pacc")
                        for ct in range(4):
                            nc.tensor.matmul(
                                acc[:],
                                wbig[:, ct, 3 * C + jb * 128:
                                     3 * C + (jb + 1) * 128],
                                att[:, ct, th * 512:(th + 1) * 512],
                                start=(ct == 0), stop=False)
                        nc.tensor.matmul(
                            acc[:], sm[0:1, SM_BFIN + jb * 128:
                                        SM_BFIN + (jb + 1) * 128],
                            ones(512), start=False, stop=True)
                        nc.vector.tensor_copy(
                            outT[:, jb, th * 512:(th + 1) * 512], acc[:])
                odv = out_d.rearrange("(t p) n -> p t n", p=128)
                for t in range(2):
                    nc.sync.dma_start(odv[:, 2 * t:2 * t + 2, :],
                                      outT[:, 2 * t:2 * t + 2, :])

            n2, tail = divmod(reps, 2)
            if n2:
                with tc.For_i(0, n2):
                    emit_rep()
                    emit_rep()
            for _ in range(tail):
                emit_rep()

    nc.compile()
    return nc


def _host_prep(inputs):
    x = np.asarray(inputs["x"], np.float32)
    Wq = np.asarray(inputs["Wq"], np.float32)
    bq = np.asarray(inputs["bq"], np.float32)
    Wkv = np.asarray(inputs["Wkv"], np.float32)
    bkv = np.asarray(inputs["bkv"], np.float32)
    Wproj = np.asarray(inputs["Wproj"], np.float32)
    bproj = np.asarray(inputs["bproj"], np.float32)
    Aq = np.asarray(inputs["Aq"], np.float32)
    Bq = np.asarray(inputs["Bq"], np.float32)
    Av = np.asarray(inputs["Av"], np.float32)
    Bv = np.asarray(inputs["Bv"], np.float32)
    Wsr = np.asarray(inputs["Wsr"], np.float32)
    bsr = np.asarray(inputs["bsr"], np.float32)
    gamma = np.asarray(inputs["gamma"], np.float32)
    beta = np.asarray(inputs["beta"], np.float32)
    scale = DH ** -0.5

    f16 = np.float16
    Wq_eff = ((Wq + Aq @ Bq) * scale).astype(f16)
    bq_eff = (bq * scale).astype(np.float32)
    AvBv = Av @ Bv
    Wk_e = Wkv[:, :C] + AvBv
    Wv_e = Wkv[:, C:] + AvBv
    Wk_g = (gamma[:, None] * Wk_e).astype(f16)
    Wv_g = (gamma[:, None] * Wv_e).astype(f16)
    cbk = (beta @ Wk_e + bkv[:C]).astype(f16)
    cbv = (beta @ Wv_e + bkv[C:]).astype(f16)
    wg1k = Wk_g.astype(np.float32).sum(0).astype(f16)
    wg1v = Wv_g.astype(np.float32).sum(0).astype(f16)
    Wsr_flat = np.ascontiguousarray(Wsr.reshape(4 * C, C)).astype(f16)

    sm = np.zeros((1, SM_LEN), f16)
    sm[0, SM_BFIN:SM_BFIN + C] = bproj.astype(f16)
    sm[0, SM_ONES:SM_ONES + 1024] = 1.0

    sm2 = np.zeros((2, 2 * C), f16)
    sm2[0, 0:C] = wg1k
    sm2[1, 0:C] = cbk
    sm2[0, C:2 * C] = wg1v
    sm2[1, C:2 * C] = cbv

    vecs = np.zeros((128, 8), np.float32)
    for cb in range(4):
        vecs[:, cb] = bsr[cb * 128:(cb + 1) * 128]
        vecs[:, 4 + cb] = bq_eff[cb * 128:(cb + 1) * 128]

    wbig = np.zeros((4, 128, 4 * C), f16)
    for ct in range(4):
        rows = slice(ct * 128, (ct + 1) * 128)
        wbig[ct, :, 0:C] = Wq_eff[rows]
        wbig[ct, :, C:2 * C] = Wk_g[rows]
        wbig[ct, :, 2 * C:3 * C] = Wv_g[rows]
        wbig[ct, :, 3 * C:4 * C] = Wproj[rows].astype(f16)

    shared = {
        "wsr": Wsr_flat.reshape(16, 128, C),
        "wbig": wbig,
        "sm": sm,
        "sm2": sm2,
        "vecs": vecs,
        "cst": np.full((128, 1), 1.0 / C, f16),
        "epsc": np.full((1, 1), LN_EPS, np.float32),
    }
    xT = [np.ascontiguousarray(x[b].T).astype(f16) for b in range(B)]
    in_maps = []
    for core in range(NCORES):
        b, qi = core // 4, core % 4
        m = dict(shared)
        m["xTs"] = xT[b]
        m["xq"] = np.ascontiguousarray(xT[b][:, qi * NQ:(qi + 1) * NQ])
        in_maps.append(m)
    return in_maps


def run_device(inputs, reps=1, phases='all'):
    from concourse.bass_utils import run_bass_kernel_spmd
    key = f"nc{reps}"
    if key not in _cached:
        _cached[key] = _build_nc(reps)
    nc = _cached[key]
    in_maps = _host_prep(inputs)
    res = run_bass_kernel_spmd(nc, in_maps, core_ids=list(range(NCORES)))
    return res


def kernel(**inputs):
    inputs = {k: np.asarray(v) for k, v in inputs.items()}
    res = run_device(inputs, reps=1)
    out = np.zeros((B, N, C), np.float32)
    for core in range(NCORES):
        b, qi = core // 4, core % 4
        out[b, qi * NQ:(qi + 1) * NQ, :] = \
            res.results[core]["outT"].astype(np.float32).T
    return out


# revision 24
# speedup vs baseline: 1.2500x; 1.2500x over previous
"""Trainium2 Bass kernel for PVT-style spatial-reduction attention with LoRA.

Sharding: 8 cores = (batch b in {0,1}) x (query-token quarter qi in {0..3}).
Each core computes the full spatial-reduction conv + LayerNorm + K/V
(replicated within a batch group) and attention + output projection for its
own 1024 query tokens. No collectives at all: the host concatenates the
per-core output slices. The whole per-rep computation sits inside a For_i
hardware loop, so multi-rep NEFFs stay the same static size as reps=1.

All activations live transposed ([feature, token]) on device. Host folds:
LoRA into the dense weights, softmax scale into Wq/bq, LN gamma into Wk/Wv.
The per-position LN shift/scale is applied as xsn = xs*rstd (matmul-broadcast
of rstd along partitions) plus rank-1 correction matmuls (wg1 x (-mu*rstd)
and beta-derived column biases) accumulated directly into the K/V PSUM
groups. Softmax denominators come from an all-ones column appended to each
head's V block; max-subtraction is skipped (logits are bounded ~|2|).
"""
import sys
for _p in ('/opt/trn_rl_repo', '/root/.axon_site/_ro/trn_rl_repo'):
    if _p not in sys.path:
        sys.path.insert(0, _p)

import numpy as np

B, N, C, HEAD, SR, R = 2, 4096, 512, 8, 2, 8
HH = WW = 64
DH = C // HEAD               # 64
M = (HH // SR) * (WW // SR)  # 1024 kv positions
NQ = N // 4                  # 1024 query tokens per core
LN_EPS = 1e-5
NCORES = 8

_cached = {}

# sm (small-vector) column layout
SM_WG1K = 0
SM_WG1V = 512
SM_CBK = 1024
SM_CBV = 1536
SM_BFIN = 2048
SM_ONES = 2560
SM_LEN = 3584


def _build_nc(reps=1):
    from concourse import bacc, tile, mybir
    import concourse.bass as bass_mod

    f32 = mybir.dt.float32
    f16 = mybir.dt.float16
    ACT = mybir.ActivationFunctionType

    nc = bacc.Bacc("TRN2", target_bir_lowering=False, debug=False,
                   num_devices=NCORES)
    xTs_d = nc.dram_tensor("xTs", [C, N], f16, kind="ExternalInput")
    xq_d = nc.dram_tensor("xq", [C, NQ], f16, kind="ExternalInput")
    wsr_d = nc.dram_tensor("wsr", [16, 128, C], f16, kind="ExternalInput")
    wbig_d = nc.dram_tensor("wbig", [4, 128, 4 * C], f16, kind="ExternalInput")
    sm_d = nc.dram_tensor("sm", [1, SM_LEN], f16, kind="ExternalInput")
    sm2_d = nc.dram_tensor("sm2", [2, 2 * C], f16, kind="ExternalInput")
    vecs_d = nc.dram_tensor("vecs", [128, 8], f32, kind="ExternalInput")
    cst_d = nc.dram_tensor("cst", [128, 2], f16, kind="ExternalInput")
    eps_d = nc.dram_tensor("epsc", [1, 1], f32, kind="ExternalInput")
    out_d = nc.dram_tensor("outT", [C, NQ], f16, kind="ExternalOutput")

    with tile.TileContext(nc) as tc:
        with tc.tile_pool(name="sb", bufs=1) as sb, \
             tc.tile_pool(name="pex", bufs=4) as pex, \
             tc.tile_pool(name="fin", bufs=2) as fin, \
             tc.tile_pool(name="dbl", bufs=2) as dbl, \
             tc.tile_pool(name="ps", bufs=1, space="PSUM") as ps:

            xT = sb.tile([128, 4, N], f16)
            wsr = sb.tile([128, 16, C], f16)
            wbig = sb.tile([128, 4, 4 * C], f16)
            sm = sb.tile([1, SM_LEN], f16)
            sm2 = sb.tile([2, 2 * C], f16)
            vecs = sb.tile([128, 8], f32)
            cst = sb.tile([128, 2], f16)
            epsc = sb.tile([1, 1], f32)
            xs = sb.tile([128, 4, M], f16)
            att = sb.tile([128, 4, NQ], f16)
            sq = sb.tile([128, 4, M], f16)
            mean_s = sb.tile([1, M], f32)
            e2_s = sb.tile([1, M], f32)
            msq_s = sb.tile([1, M], f32)
            rstd16 = sb.tile([1, M], f16)
            stat2 = sb.tile([2, M], f16)
            xsn = sb.tile([128, 4, M], f16)
            outT = sb.tile([128, 4, NQ], f16)
            on64 = sb.tile([65, 64], f16)

            ones = lambda n: sm[0:1, SM_ONES:SM_ONES + n]

            # ---- weights / constants: loaded once, stationary across reps --
            nc.sync.dma_start(wsr[:], wsr_d.rearrange("g p n -> p g n"))
            nc.sync.dma_start(wbig[:], wbig_d.rearrange("t p n -> p t n"))
            nc.sync.dma_start(sm[:], sm_d[:])
            nc.sync.dma_start(sm2[:], sm2_d[:])
            nc.sync.dma_start(vecs[:], vecs_d[:])
            nc.sync.dma_start(cst[:], cst_d[:])
            nc.sync.dma_start(epsc[:], eps_d[:])
            nc.vector.memset(on64[64:65, :], 1.0)
            nc.sync.dma_start(stat2[1:2, :],
                              sm_d[0:1, SM_ONES:SM_ONES + M])

            def emit_rep():
                xq = dbl.tile([128, 4, NQ], f16, tag="xq")
                kT = dbl.tile([128, 4, M], f16, tag="kT")
                vt = dbl.tile([128, 8, 520], f16, tag="vt")
                qT = dbl.tile([128, 4, NQ], f16, tag="qT")
                # ---- load activations (split across DMA queues) ----
                xqv = xq_d.rearrange("(t p) n -> p t n", p=128)
                for t in range(2):
                    nc.sync.dma_start(xq[:, 2 * t:2 * t + 2, :],
                                      xqv[:, 2 * t:2 * t + 2, :])
                xTv = xTs_d.rearrange("(t p) n -> p t n", p=128)
                for t in range(4):
                    for hl in range(2):
                        nc.sync.dma_start(
                            xT[:, t, hl * 2048:(hl + 1) * 2048],
                            xTv[:, t, hl * 2048:(hl + 1) * 2048])

                # ---- qT [C, NQ] (this core's token quarter; input lands
                # first, keeps PE busy while xT streams in) ----
                for jb in range(4):
                    for th in range(2):
                        acc = ps.tile([128, 512], f32, tag="nrw", bufs=4,
                                      name="qacc")
                        for ct in range(4):
                            nc.tensor.matmul(
                                acc[:], wbig[:, ct, jb * 128:(jb + 1) * 128],
                                xq[:, ct, th * 512:(th + 1) * 512],
                                start=(ct == 0), stop=(ct == 3))
                        nc.vector.tensor_scalar_add(
                            qT[:, jb, th * 512:(th + 1) * 512], acc[:],
                            vecs[:, 4 + jb:5 + jb])

                # ---- conv: xs_pre^T [C, M] ----
                xview = xT.rearrange("p t (ph a pw b) -> p t ph a pw b",
                                     ph=32, a=2, pw=32, b=2)
                for cb in range(4):
                    for mc in range(2):
                        acc = ps.tile([128, 512], f32, tag="nrw", bufs=4,
                                      name="cacc")
                        for g in range(16):
                            dydx, ct = g // 4, g % 4
                            dy, dx = dydx // 2, dydx % 2
                            rhs = xview[:, ct, mc * 16:(mc + 1) * 16, dy, :, dx]
                            nc.tensor.matmul(
                                acc[:], wsr[:, g, cb * 128:(cb + 1) * 128],
                                rhs, start=(g == 0), stop=(g == 15))
                        nc.vector.tensor_scalar_add(
                            xs[:, cb, mc * 512:(mc + 1) * 512], acc[:],
                            vecs[:, cb:cb + 1])

                # ---- LN stats ----
                nc.vector.tensor_mul(sq[:], xs[:], xs[:])
                for ci, (srcT, dst) in enumerate(((xs, mean_s), (sq, e2_s))):
                    for mh in range(2):
                        st = ps.tile([1, 512], f32, tag="nrw", bufs=4,
                                     name="st")
                        for cb in range(4):
                            nc.tensor.matmul(
                                st[:], cst[:, ci:ci + 1],
                                srcT[:, cb, mh * 512:(mh + 1) * 512],
                                start=(cb == 0), stop=(cb == 3))
                        nc.vector.tensor_copy(
                            dst[0:1, mh * 512:(mh + 1) * 512], st[:])
                nc.vector.tensor_mul(msq_s[:], mean_s[:], mean_s[:])
                nc.vector.tensor_sub(e2_s[:], e2_s[:], msq_s[:])
                nc.scalar.activation(out=e2_s[:], in_=e2_s[:], func=ACT.Sqrt,
                                     bias=epsc[0:1, 0:1], scale=1.0)
                nc.vector.reciprocal(e2_s[:], e2_s[:])       # rstd
                # mean_s holds -mean (cst = -1/C), so this is -mu*rstd
                nc.vector.tensor_mul(mean_s[:], mean_s[:], e2_s[:])
                nc.vector.tensor_copy(rstd16[:], e2_s[:])
                nc.vector.tensor_copy(stat2[0:1, :], mean_s[:])

                # xsn = xs * rstd (broadcast rstd along partitions via matmul)
                rbc = ps.tile([128, M], f32, tag="wide", bufs=2, name="rbc")
                for mh in range(2):
                    nc.tensor.matmul(rbc[:, mh * 512:(mh + 1) * 512],
                                     ones(128),
                                     rstd16[0:1, mh * 512:(mh + 1) * 512],
                                     start=True, stop=True)
                for cb in range(4):
                    nc.vector.tensor_mul(xsn[:, cb, :], xs[:, cb, :], rbc[:])

                # ---- kT [C, M] ----
                for jb in range(4):
                    for mh in range(2):
                        acc = ps.tile([128, 512], f32, tag="nrw", bufs=4,
                                      name="kacc")
                        for ct in range(4):
                            nc.tensor.matmul(
                                acc[:],
                                wbig[:, ct, C + jb * 128:C + (jb + 1) * 128],
                                xsn[:, ct, mh * 512:(mh + 1) * 512],
                                start=(ct == 0), stop=False)
                        nc.tensor.matmul(
                            acc[:], sm2[0:2, jb * 128:(jb + 1) * 128],
                            stat2[0:2, mh * 512:(mh + 1) * 512],
                            start=False, stop=True)
                        nc.vector.tensor_copy(
                            kT[:, jb, mh * 512:(mh + 1) * 512], acc[:])

                # ---- v [M, C] + per-head ones column ----
                nc.vector.memset(bass_mod.AP(
                    tensor=vt.tensor, offset=vt.offset + 64,
                    ap=[list(vt.ap[0]), [520, 8], [65, 8]]), 1.0)
                for kt in range(8):
                    acc = ps.tile([128, 512], f32, tag="nrw", bufs=4,
                                  name="vacc")
                    for ct in range(4):
                        nc.tensor.matmul(
                            acc[:], xsn[:, ct, kt * 128:(kt + 1) * 128],
                            wbig[:, ct, 2 * C:3 * C],
                            start=(ct == 0), stop=False)
                    nc.tensor.matmul(
                        acc[:], stat2[0:2, kt * 128:(kt + 1) * 128],
                        sm2[0:2, C:C + 512],
                        start=False, stop=True)
                    vdst = bass_mod.AP(tensor=vt.tensor,
                                       offset=vt.offset + kt * 520,
                                       ap=[list(vt.ap[0]), [65, 8], [1, 64]])
                    nc.vector.tensor_copy(
                        vdst, acc.rearrange("p (h d) -> p h d", h=8))

                # ---- attention ----
                for th in range(2):
                    for h in range(8):
                        hb, ho = h // 2, 64 * (h % 2)
                        ops = ps.tile([65, 512], f32, tag="nrw", bufs=4,
                                      name="ops")
                        # scores + exp for all chunks first, then the av
                        # accumulation: keeps independent score matmuls ahead
                        # of exp-dependent av matmuls in the PE queue.
                        pexps = []
                        for mc2 in range(4):
                            sps = ps.tile([128, 1024], f32, tag="wide",
                                          bufs=2, name="sps")
                            for half in range(2):
                                mc = 2 * mc2 + half
                                nc.tensor.matmul(
                                    sps[:, half * 512:(half + 1) * 512],
                                    kT[ho:ho + 64, hb,
                                       mc * 128:(mc + 1) * 128],
                                    qT[ho:ho + 64, hb,
                                       th * 512:(th + 1) * 512],
                                    start=True, stop=True)
                            pexp = pex.tile([128, 1024], f16, tag="pexp")
                            nc.scalar.activation(out=pexp[:], in_=sps[:],
                                                 func=ACT.Exp)
                            pexps.append(pexp)
                        for mc2 in range(4):
                            for half in range(2):
                                mc = 2 * mc2 + half
                                nc.tensor.matmul(
                                    ops[:], vt[:, mc, 65 * h:65 * h + 65],
                                    pexps[mc2][:, half * 512:(half + 1) * 512],
                                    start=(mc == 0), stop=(mc == 7))
                        rbt = fin.tile([65, 512], f16, tag="rbt", name="rbt")
                        with nc.allow_low_precision(reason="f16 recip ok"):
                            nc.vector.reciprocal(rbt[64:65, :], ops[64:65, :])
                        bcd = ps.tile([64, 512], f32, tag="nrw", bufs=4,
                                      name="bcd")
                        nc.tensor.matmul(bcd[:], on64[64:65, :],
                                         rbt[64:65, :],
                                         start=True, stop=True)
                        num = fin.tile([64, 512], f16, tag="num", name="num")
                        nc.vector.tensor_copy(num[:], ops[0:64, :])
                        nc.vector.tensor_mul(
                            att[ho:ho + 64, hb, th * 512:(th + 1) * 512],
                            num[:], bcd[:])

                    # ---- out proj + store for this token half ----
                    for jb in range(4):
                        acc = ps.tile([128, 512], f32, tag="nrw", bufs=4,
                                      name="pacc")
                        for ct in range(4):
                            nc.tensor.matmul(
                                acc[:],
                                wbig[:, ct, 3 * C + jb * 128:
                                     3 * C + (jb + 1) * 128],
                                att[:, ct, th * 512:(th + 1) * 512],
                                start=(ct == 0), stop=False)
                        nc.tensor.matmul(
                            acc[:], sm[0:1, SM_BFIN + jb * 128:
                                        SM_BFIN + (jb + 1) * 128],
                            ones(512), start=False, stop=True)
                        nc.vector.tensor_copy(
                            outT[:, jb, th * 512:(th + 1) * 512], acc[:])
                    odv = out_d.rearrange("(t p) n -> p t n", p=128)
                    for tb in range(2):
                        nc.sync.dma_start(
                            odv[:, 2 * tb:2 * tb + 2,
                                th * 512:(th + 1) * 512],
                            outT[:, 2 * tb:2 * tb + 2,
                                 th * 512:(th + 1) * 512])

            n2, tail = divmod(reps, 2)
            if n2:
                with tc.For_i(0, n2):
                    emit_rep()
                    emit_rep()
            for _ in range(tail):
                emit_rep()

    nc.compile()
    return nc


def _host_prep(inputs):
    x = np.asarray(inputs["x"], np.float32)
    Wq = np.asarray(inputs["Wq"], np.float32)
    bq = np.asarray(inputs["bq"], np.float32)
    Wkv = np.asarray(inputs["Wkv"], np.float32)
    bkv = np.asarray(inputs["bkv"], np.float32)
    Wproj = np.asarray(inputs["Wproj"], np.float32)
    bproj = np.asarray(inputs["bproj"], np.float32)
    Aq = np.asarray(inputs["Aq"], np.float32)
    Bq = np.asarray(inputs["Bq"], np.float32)
    Av = np.asarray(inputs["Av"], np.float32)
    Bv = np.asarray(inputs["Bv"], np.float32)
    Wsr = np.asarray(inputs["Wsr"], np.float32)
    bsr = np.asarray(inputs["bsr"], np.float32)
    gamma = np.asarray(inputs["gamma"], np.float32)
    beta = np.asarray(inputs["beta"], np.float32)
    scale = DH ** -0.5

    f16 = np.float16
    Wq_eff = ((Wq + Aq @ Bq) * scale).astype(f16)
    bq_eff = (bq * scale).astype(np.float32)
    AvBv = Av @ Bv
    Wk_e = Wkv[:, :C] + AvBv
    Wv_e = Wkv[:, C:] + AvBv
    Wk_g = (gamma[:, None] * Wk_e).astype(f16)
    Wv_g = (gamma[:, None] * Wv_e).astype(f16)
    cbk = (beta @ Wk_e + bkv[:C]).astype(f16)
    cbv = (beta @ Wv_e + bkv[C:]).astype(f16)
    wg1k = Wk_g.astype(np.float32).sum(0).astype(f16)
    wg1v = Wv_g.astype(np.float32).sum(0).astype(f16)
    Wsr_flat = np.ascontiguousarray(Wsr.reshape(4 * C, C)).astype(f16)

    sm = np.zeros((1, SM_LEN), f16)
    sm[0, SM_BFIN:SM_BFIN + C] = bproj.astype(f16)
    sm[0, SM_ONES:SM_ONES + 1024] = 1.0

    sm2 = np.zeros((2, 2 * C), f16)
    sm2[0, 0:C] = wg1k
    sm2[1, 0:C] = cbk
    sm2[0, C:2 * C] = wg1v
    sm2[1, C:2 * C] = cbv

    vecs = np.zeros((128, 8), np.float32)
    for cb in range(4):
        vecs[:, cb] = bsr[cb * 128:(cb + 1) * 128]
        vecs[:, 4 + cb] = bq_eff[cb * 128:(cb + 1) * 128]

    wbig = np.zeros((4, 128, 4 * C), f16)
    for ct in range(4):
        rows = slice(ct * 128, (ct + 1) * 128)
        wbig[ct, :, 0:C] = Wq_eff[rows]
        wbig[ct, :, C:2 * C] = Wk_g[rows]
        wbig[ct, :, 2 * C:3 * C] = Wv_g[rows]
        wbig[ct, :, 3 * C:4 * C] = Wproj[rows].astype(f16)

    shared = {
        "wsr": Wsr_flat.reshape(16, 128, C),
        "wbig": wbig,
        "sm": sm,
        "sm2": sm2,
        "vecs": vecs,
        "cst": np.stack([np.full(128, -1.0 / C, f16),
                         np.full(128, 1.0 / C, f16)], axis=1),
        "epsc": np.full((1, 1), LN_EPS, np.float32),
    }
    xT = [np.ascontiguousarray(x[b].T).astype(f16) for b in range(B)]
    in_maps = []
    for core in range(NCORES):
        b, qi = core // 4, core % 4
        m = dict(shared)
        m["xTs"] = xT[b]
        m["xq"] = np.ascontiguousarray(xT[b][:, qi * NQ:(qi + 1) * NQ])
        in_maps.append(m)
    return in_maps


def run_device(inputs, reps=1, phases='all'):
    from concourse.bass_utils import run_bass_kernel_spmd
    key = f"nc{reps}"
    if key not in _cached:
        _cached[key] = _build_nc(reps)
    nc = _cached[key]
    in_maps = _host_prep(inputs)
    res = run_bass_kernel_spmd(nc, in_maps, core_ids=list(range(NCORES)))
    return res


def kernel(**inputs):
    inputs = {k: np.asarray(v) for k, v in inputs.items()}
    res = run_device(inputs, reps=1)
    out = np.zeros((B, N, C), np.float32)
    for core in range(NCORES):
        b, qi = core // 4, core % 4
        out[b, qi * NQ:(qi + 1) * NQ, :] = \
            res.results[core]["outT"].astype(np.float32).T
    return out
